# revision 32
# baseline (speedup 1.0000x reference)
"""Trainium2 Bass kernel for nn_Attn attention-context module.

Computation (per batch b):
    enc_att = enc @ W_enc + b_enc                      # [S, A]
    dec_att = dec @ W_dec + b_dec                      # [A]
    scores  = tanh(enc_att + dec_att) @ W_att + b_att  # [S]
    w       = softmax(mask(scores))                    # over S
    out     = sum_s w[s] * enc_att[s]                  # [A]

Strategy: data-parallel over batch across 8 NeuronCores (4 batches each),
weights replicated.

Masked tokens contribute exactly zero to the softmax numerator, denominator
and context (their score gets -32768 folded in, and exp underflows to +0), so
the host compacts each batch to its unmasked tokens, padded with zeroed,
masked-out tokens up to a global per-batch token count Kp (multiple of 64,
shared by all batches so the 8 cores run one instruction stream). With the
reference's ~50% mask density this roughly halves all device work. The
compacted enc is cast to bf16 and pre-transposed on the host so each core
streams contiguous, already-transposed bf16 tiles straight from HBM.

Device schedule: the token stream is cut into 512-wide PE tiles; the sub-512
tails of all batch positions are packed side by side into shared multi-segment
tiles so the PE never runs skinny matmuls (each tile = 32 dense N<=512 MMs).

Per tile:
  - PE computes enc_attT chunks [A-chunk(128), w tok] in PSUM (bf16 in,
    fp32 acc)
  - ACT applies tanh (bf16 out) with per-partition bias = dec_att + b_enc
    (+ b_dec), per segment; raw enc_att is copied to SBUF bf16 (ACT and DVE
    split the 4 chunks) for the context accumulation
  - scores: DVE premultiplies tanh by W_att per A-chunk and accumulates
    across the 4 chunks (tensor_scalar + 3 scalar_tensor_tensor, bf16), then
    ONE K=128 PE matmul with an all-ones lhsT reduces across partitions and
    broadcasts the score row to all 128 partitions of a PSUM tile; the mask
    is folded in as a -32768*mask K=1 matmul term (exp underflows to 0,
    killing masked + padding tokens)
  - softmax without max-subtraction (|scores| <= ||W_att||_1 ~ 51, exp can't
    overflow fp32; b_att cancels in the softmax so it is dropped); exp runs
    on the broadcast PSUM scores per segment, yielding bf16 numerators
    already replicated across partitions and per-partition denominators via
    accum_out (so no broadcast matmuls are needed anywhere downstream)
  - context accumulated per segment with fused DVE multiply+row-sum
    (scalar_tensor_tensor with accum_out, all-bf16 operands for 2x DVE);
    normalization and b_enc are applied once per batch (reciprocal is
    per-partition, again no broadcast needed)
"""

import os
import sys

import numpy as np

for _p in ("/opt/trn_rl_repo", "/root/.axon_site/_ro/trn_rl_repo"):
    if os.path.isdir(_p) and _p not in sys.path:
        sys.path.append(_p)

import concourse.bass as bass
import bass_rust
import concourse.mybir as mybir
from concourse import tile
from concourse.bass_utils import run_bass_kernel_spmd

P = 128
E = 1024          # 2*HIDDEN
A = 512           # ATT
HID = 512
S = 2048
B = 32
NCORES = 8
BLOC = B // NCORES           # 4 batches per core
TT = 512                     # tokens per PE tile
NE = E // P                  # 8 E-chunks
NA = A // P                  # 4 A-chunks

f32 = mybir.dt.float32
bf16 = mybir.dt.bfloat16
u8 = mybir.dt.uint8

_CACHE = {}


GRAN = 16  # padding granularity (matmul N is arbitrary)


def make_schedule(kps, kmins):
    """Cut the padded per-batch token streams into PE tiles.

    Full 512-wide tiles are single-segment; the sub-512 tails of all batch
    positions are packed side by side into shared multi-segment tiles.
    Returns (tiles, nslots, needs_mask, last_tile):
      tiles[ti]    = [(b, slot, off, w), ...]
      nslots[b]    = number of (b, slot) pairs for batch position b
      needs_mask[ti] = tile contains masked or padding tokens on some core
      last_tile[b] = index of the tile holding b's last slot
    """
    tiles, tails, nslots = [], [], []
    for b, kp in enumerate(kps):
        nf = kp // TT
        for t in range(nf):
            tiles.append([(b, t, 0, TT)])
        if kp % TT:
            tails.append((b, nf, kp % TT))
        nslots.append(nf + (1 if kp % TT else 0))
    cur, curw = [], 0
    for b, slot, w in tails:
        if curw + w > TT:
            tiles.append(cur)
            cur, curw = [], 0
        cur.append((b, slot, curw, w))
        curw += w
    if cur:
        tiles.append(cur)
    needs_mask = [
        any(slot * TT + w > kmins[b] for (b, slot, off, w) in segs)
        for segs in tiles
    ]
    last_tile = [max(ti for ti, segs in enumerate(tiles)
                     if any(s[0] == b for s in segs)) for b in range(BLOC)]
    return tiles, nslots, needs_mask, last_tile


def _split_multiwaits(nc):
    """This toolchain's walrus encodes at most 1 sync-wait per instruction
    (2 for EventSemaphore). Hoist extra waits onto pure-wait EventSemaphore
    instructions inserted immediately before the offender (same engine), which
    preserves semantics exactly."""
    n_split = 0
    uid = 0
    for fn in nc.m.functions:
        for blk in fn.blocks:
            new_insts = []
            for inst in blk.instructions:
                cap = 2 if type(inst).__name__ == "InstEventSemaphore" else 1
                si = inst.sync_info
                waits = list(si.on_wait) if si is not None and si.on_wait else []
                if len(waits) > cap:
                    extra, keep = waits[:-cap], waits[-cap:]
                    for i in range(0, len(extra), 2):
                        uid += 1
                        new_insts.append(bass_rust.InstEventSemaphore(
                            name=f"splitwait_{uid}_{inst.name}",
                            engine=inst.engine,
                            ins=[],
                            outs=[],
                            sync_info=bass_rust.SyncInfo(
                                on_wait=list(extra[i:i + 2]), on_update=[]),
                        ))
                        n_split += 1
                    si.on_wait = keep
                new_insts.append(inst)
            blk.instructions[:] = new_insts
    return n_split


def build(kps=None, kmins=None, encbufs=4, tebufs=4, eabufs=5,
          reps=1, probe=None, dmaq=2, scores_pe=False):
    if kps is None:
        kps = [S] * BLOC
    if kmins is None:
        kmins = [0] * BLOC
    tiles, nslots, needs_mask, last_tile = make_schedule(kps, kmins)
    NTILE = len(tiles)
    NSLOT = max(nslots)

    nc = bass.Bass("TRN2", debug=False)
    # host-compacted, pre-transposed bf16 enc in tile layout, partition-major
    # so a full tile is one fully-contiguous 8 KiB-per-partition DMA:
    # [ti, p, i, n] = enc_tile[ti, n, i*P+p]
    encT = nc.dram_tensor("encT", [NTILE, P, NE, TT], bf16,
                          kind="ExternalInput")
    # decoder_hidden, host-transposed: [p, hc*BLOC + b] = dec[b, hc*P + p]
    decT = nc.dram_tensor("decT", [P, (HID // P) * BLOC], bf16,
                          kind="ExternalInput")
    # compacted masks in tile layout, padded region = 1: [ti*TT + n]
    masks = nc.dram_tensor("masks", [NTILE * TT], u8, kind="ExternalInput")
    w_enc = nc.dram_tensor("w_enc", [E, A], bf16, kind="ExternalInput")
    b_enc = nc.dram_tensor("b_enc", [A], f32, kind="ExternalInput")
    w_dec = nc.dram_tensor("w_dec", [HID, A], bf16, kind="ExternalInput")
    b_dec = nc.dram_tensor("b_dec", [A], f32, kind="ExternalInput")
    w_att = nc.dram_tensor("w_att", [A], f32, kind="ExternalInput")
    out = nc.dram_tensor("out", [BLOC, A], f32, kind="ExternalOutput")

    Tanh = mybir.ActivationFunctionType.Tanh
    Exp = mybir.ActivationFunctionType.Exp
    Copy = mybir.ActivationFunctionType.Copy
    add = mybir.AluOpType.add
    mult = mybir.AluOpType.mult
    X = mybir.AxisListType.X

    with tile.TileContext(nc) as tc:
        with (
            tc.tile_pool(name="const", bufs=1) as cp,
            tc.tile_pool(name="encT", bufs=encbufs) as encp,
            tc.tile_pool(name="tanh", bufs=tebufs) as tanhp,
            tc.tile_pool(name="ea", bufs=eabufs) as eap,
            tc.tile_pool(name="wt", bufs=4) as wtp,
            tc.tile_pool(name="small", bufs=3) as smp,
            tc.tile_pool(name="attps", bufs=2, space="PSUM") as attp,
            tc.tile_pool(name="scps", bufs=2, space="PSUM") as scp,
        ):
            # ---------------- one-time prep ----------------
            # W_enc bf16: [e' part, (i, a)] for e = i*128 + e'
            wsb = cp.tile([P, NE * A], bf16, tag="wsb")
            nc.gpsimd.dma_start(
                wsb[:].rearrange("p (i a) -> p i a", i=NE),
                w_enc.ap().rearrange("(i p) a -> p i a", p=P))
            # W_dec bf16: [h' part, (i, a)] for h = i*128 + h'
            wdsb = cp.tile([P, (HID // P) * A], bf16, tag="wdsb")
            nc.scalar.dma_start(
                wdsb[:].rearrange("p (i a) -> p i a", i=HID // P),
                w_dec.ap().rearrange("(i p) a -> p i a", p=P))
            # W_att f32 column chunks [a' part, j] (per-partition STT scalar)
            wasf = cp.tile([P, NA], f32, tag="wasf")
            nc.scalar.dma_start(wasf[:], w_att.ap().rearrange("(j p) -> p j", p=P))
            # biases as column chunks [a' part, j]
            besb = cp.tile([P, NA], f32, tag="besb")
            nc.scalar.dma_start(besb[:], b_enc.ap().rearrange("(j p) -> p j", p=P))
            bdsb = cp.tile([P, NA], f32, tag="bdsb")
            nc.scalar.dma_start(bdsb[:], b_dec.ap().rearrange("(j p) -> p j", p=P))
            bbsb = cp.tile([P, NA], f32, tag="bbsb")
            nc.vector.tensor_tensor(bbsb[:], besb[:], bdsb[:], op=add)
            # decoder_hidden transposed [h' part, (hc, b)] (host-prepped)
            dhT = cp.tile([P, (HID // P) * BLOC], bf16, tag="dhT")
            nc.scalar.dma_start(dhT[:], decT.ap())
            # masks, whole core's worth: [1, NTILE*TT] u8 -> bf16
            msku = cp.tile([1, NTILE * TT], u8, tag="msku")
            nc.gpsimd.dma_start(msku[:], masks.ap()[None, :])
            mskf = cp.tile([1, NTILE * TT], bf16, tag="mskf")
            nc.vector.tensor_copy(mskf[:], msku[:])
            # all-ones lhsT: one K=128 matmul both reduces the premultiplied
            # tanh across partitions and broadcasts the scores to M=128
            ones128 = cp.tile([P, P], bf16, tag="ones128")
            nc.vector.memset(ones128[:], 1.0)
            # replicated W_att lhsT chunks (scores_pe variant): every output
            # column m of chunk j gets W_att[j*128+p], so the score matmul
            # reduces AND broadcasts without any DVE premultiply
            wrep = cp.tile([P, NA * P], bf16, tag="wrep")
            for j in range(NA):
                nc.vector.tensor_scalar(
                    out=wrep[:, j * P:(j + 1) * P], in0=ones128[:],
                    scalar1=wasf[:, j:j + 1], scalar2=None, op0=mult)
            # mask weight row for the -32768*mask K=1 broadcast matmul term
            m30row = cp.tile([1, P], bf16, tag="m30row")
            nc.vector.memset(m30row[:], -32768.0)

            # dec_attT + bias columns: bias_sb[a', j*BLOC + b]
            bias_sb = cp.tile([P, NA * BLOC], f32, tag="bias_sb")
            for j in range(NA):
                pd = scp.tile([P, BLOC], f32, tag="sc")
                for hc in range(HID // P):
                    nc.tensor.matmul(
                        pd[:],
                        lhsT=wdsb[:, hc * A + j * P: hc * A + (j + 1) * P],
                        rhs=dhT[:, hc * BLOC:(hc + 1) * BLOC],
                        start=(hc == 0), stop=(hc == HID // P - 1))
                nc.vector.tensor_scalar(
                    out=bias_sb[:, j * BLOC:(j + 1) * BLOC], in0=pd[:],
                    scalar1=bbsb[:, j:j + 1], scalar2=None, op0=add)

            # persistent accumulators (per-partition broadcast copies)
            ctxp = cp.tile([P, NA * BLOC * NSLOT], f32, tag="ctxp")
            ctxs = cp.tile([P, NA * BLOC], f32, tag="ctxs")
            densP = cp.tile([P, BLOC * NSLOT], f32, tag="densP")
            dentP = cp.tile([P, BLOC], f32, tag="dentP")
            recP = cp.tile([P, BLOC], f32, tag="recP")
            outsb = cp.tile([P, NA * BLOC], f32, tag="outsb")

            # ---------------- main loop ----------------
            # Two-stage deferred epilogue: at iteration t, the DVE premult of
            # tile t-1 is issued BEFORE tile t's main matmuls (so it executes
            # while the PE streams tile t), and the finish stage (reduce
            # matmul -> exp -> context) is issued after them (so its PE
            # matmul lands behind dense main work with its DVE input long
            # done). enc_att chunks j0/j1 are copied to SBUF bf16; j2/j3 stay
            # in PSUM and the context reads them there, saving two copies —
            # their banks (attB, 4 bufs) recycle two tiles later, after the
            # context STT has consumed them.
            premult_q, finish_q = [], []

            def premult_stage(ti, wt_, tanh_sb):
                if probe in ("dma", "mains", "tanhonly", "noscore") or scores_pe:
                    return None
                # wt = sum_j W_att_j * tanh_j   (bf16 chain on DVE)
                prev = wtp.tile([P, TT], bf16, tag="wt")
                nc.vector.tensor_scalar(
                    out=prev[:, :wt_], in0=tanh_sb[:, 0:wt_],
                    scalar1=wasf[:, 0:1], scalar2=None, op0=mult)
                for j in range(1, NA):
                    nxt = wtp.tile([P, TT], bf16, tag="wt")
                    nc.vector.scalar_tensor_tensor(
                        out=nxt[:, :wt_], in0=tanh_sb[:, j * wt_:(j + 1) * wt_],
                        scalar=wasf[:, j:j + 1], in1=prev[:, :wt_],
                        op0=mult, op1=add)
                    prev = nxt
                return prev

            def finish_stage(ti, segs, wt_, wt_tile, ea_sb, att23, tanh_sb):
                if probe in ("dma", "mains", "tanhonly", "noscore"):
                    return
                # broadcast scores [128, w]: reduce premult across partitions
                sc = scp.tile([P, TT], f32, tag="sc")
                nm = needs_mask[ti]
                if scores_pe:
                    for j in range(NA):
                        nc.tensor.matmul(
                            sc[:, :wt_], lhsT=wrep[:, j * P:(j + 1) * P],
                            rhs=tanh_sb[:, j * wt_:(j + 1) * wt_],
                            start=(j == 0), stop=(j == NA - 1 and not nm))
                else:
                    nc.tensor.matmul(sc[:, :wt_], lhsT=ones128[:],
                                     rhs=wt_tile[:, :wt_], start=True,
                                     stop=not nm)
                if nm:
                    nc.tensor.matmul(
                        sc[:, :wt_], lhsT=m30row[:],
                        rhs=mskf[0:1, ti * TT: ti * TT + wt_],
                        start=False, stop=True)
                # exp per segment (per-partition denominators via accum_out)
                p_sb = smp.tile([P, TT], bf16, tag="p_sb")
                for (b, slot, off, w) in segs:
                    bt = b * NSLOT + slot
                    nc.scalar.activation(
                        p_sb[:, off:off + w], sc[:, off:off + w], Exp,
                        accum_out=densP[:, bt:bt + 1])
                if probe == "noctx":
                    return
                # fused context accumulation per (A-chunk, segment):
                # accum_out = sum_tok(p * enc_att) per partition.
                # j2/j3 first — they read (and free) the PSUM att banks.
                waste = smp.tile([P, TT], bf16, tag="waste")
                for j in (2, 3, 0, 1):
                    for (b, slot, off, w) in segs:
                        src = (att23[j - 2][:, off:off + w] if j >= 2 else
                               ea_sb[:, j * TT + off: j * TT + off + w])
                        nc.vector.scalar_tensor_tensor(
                            out=waste[:, off:off + w],
                            in0=p_sb[:, off:off + w], scalar=1.0,
                            in1=src, op0=mult, op1=mult,
                            accum_out=ctxp[:, (j * BLOC + b) * NSLOT + slot:
                                           (j * BLOC + b) * NSLOT + slot + 1])

            def batch_epilogue(b):
                if probe in ("dma", "mains", "tanhonly", "noscore", "noctx"):
                    return
                ns = nslots[b]
                # out[b] = ctx/den + b_enc (everything already per-partition)
                nc.vector.reduce_sum(
                    dentP[:, b:b + 1], densP[:, b * NSLOT:b * NSLOT + ns],
                    axis=X)
                nc.vector.reciprocal(recP[:, b:b + 1], dentP[:, b:b + 1])
                nc.vector.reduce_sum(
                    ctxs[:, b * NA:(b + 1) * NA],
                    ctxp[:].rearrange("p (j bb t) -> p j bb t", j=NA, bb=BLOC)
                    [:, :, b, :ns], axis=X)
                nc.vector.scalar_tensor_tensor(
                    out=outsb[:, b * NA:(b + 1) * NA],
                    in0=ctxs[:, b * NA:(b + 1) * NA],
                    scalar=recP[:, b:b + 1], in1=besb[:], op0=mult, op1=add)
                nc.scalar.dma_start(
                    out.ap()[b].rearrange("(j p) -> p j", p=P),
                    outsb[:, b * NA:(b + 1) * NA])

            for ti, segs in [(t_, s_) for _ in range(reps)
                             for t_, s_ in enumerate(tiles)]:
                wt_ = sum(s[3] for s in segs)
                if premult_q:
                    premult_q.pop(0)()
                # load pre-transposed tile: encTt[e', (i, n)], n < w
                encTt = encp.tile([P, NE * TT], bf16, tag="encT")
                dmae = nc.sync if (ti % 2 == 0 or dmaq == 1) else nc.scalar
                if wt_ == TT:
                    dmae.dma_start(
                        encTt[:], encT.ap()[ti].rearrange("p i n -> p (i n)"))
                else:
                    dmae.dma_start(
                        encTt[:].rearrange("p (i n) -> p i n", i=NE)
                        [:, :, :wt_],
                        encT.ap()[ti][:, :, :wt_])

                tanh_sb = tanhp.tile([P, NA * TT], bf16, tag="tanh")
                ea_sb = eap.tile([P, 2 * TT], bf16, tag="ea")
                att23 = []
                for j in range(NA if probe != "dma" else 0):
                    if j == 2 and finish_q and wt_ >= 384:
                        # previous tile's finish stage lands mid-tile: its
                        # reduce matmul slots between dense j1/j2 groups with
                        # its premult input long done, exp fires mid-tile,
                        # and the attB banks recycle with margin to spare
                        while finish_q:
                            finish_q.pop(0)()
                    if j < 2:
                        att = attp.tile([P, TT], f32, tag="attA")
                    else:
                        att = attp.tile([P, TT], f32, tag="attB", bufs=4)
                        att23.append(att)
                    for i in range(NE):
                        nc.tensor.matmul(
                            att[:, :wt_],
                            lhsT=wsb[:, i * A + j * P: i * A + (j + 1) * P],
                            rhs=encTt[:, i * TT:i * TT + wt_],
                            start=(i == 0), stop=(i == NE - 1))
                    if probe == "mains":
                        continue
                    for (b, slot, off, w) in segs:
                        nc.scalar.activation(
                            tanh_sb[:, j * wt_ + off: j * wt_ + off + w],
                            att[:, off:off + w], Tanh,
                            bias=bias_sb[:, j * BLOC + b: j * BLOC + b + 1])
                    # copy raw enc_att j0/j1 out of PSUM (ACT); j2/j3 stay
                    # in PSUM for the context and free their banks there.
                    if probe == "tanhonly" or j >= 2:
                        continue
                    nc.scalar.activation(
                        ea_sb[:, j * TT:j * TT + wt_], att[:, :wt_], Copy)

                while finish_q:  # short tiles: pop at end instead
                    finish_q.pop(0)()
                wt_holder = []
                premult_q.append(
                    lambda ti=ti, wt_=wt_, ts=tanh_sb, h=wt_holder:
                    h.append(premult_stage(ti, wt_, ts)))
                finish_q.append(
                    lambda ti=ti, segs=segs, wt_=wt_, h=wt_holder, a=ea_sb,
                    a23=att23, ts=tanh_sb:
                    finish_stage(ti, segs, wt_, h[0], a, a23, ts))
                for b in range(BLOC):
                    if last_tile[b] == ti:
                        finish_q.append(lambda b=b: batch_epilogue(b))
            while premult_q:
                premult_q.pop(0)()
            while finish_q:
                finish_q.pop(0)()

    n = _split_multiwaits(nc)
    if os.environ.get("KERNEL_DEBUG"):
        print(f"[kernel] split {n} extra waits", file=sys.stderr)
    return nc


def _get_nc(kps, kmins):
    key = (tuple(kps), tuple(kmins))
    if key not in _CACHE:
        _CACHE[key] = build(kps=list(kps), kmins=list(kmins))
    return _CACHE[key]


def plan_from_masks(masks):
    """Load-balanced batch -> (core, position) assignment. Batches are sorted
    by unmasked count (descending) and dealt position-major, so each of the
    BLOC schedule positions covers 8 similarly-sized batches and its padded
    width is the max over those 8. Returns (order, kps, kmins) where
    order[pos * NCORES + core] = original batch index."""
    m = np.asarray(masks).reshape(B, S).astype(bool)
    k = (~m).sum(axis=1).astype(int)
    order = np.argsort(-k, kind="stable")
    kps, kmins = [], []
    for pos in range(BLOC):
        grp = order[pos * NCORES:(pos + 1) * NCORES]
        kmax = max(max(int(k[g]) for g in grp), GRAN)
        kps.append(-(-kmax // GRAN) * GRAN)
        kmins.append(min(int(k[g]) for g in grp))
    return order, kps, kmins


def host_prep(enc_output, decoder_hidden, masks, W_enc, b_enc, W_dec, b_dec,
              W_att, b_att=None, **kwargs):
    """Shard + lay out inputs for the 8 cores: assign batches to (core,
    position) by unmasked count, compact each batch to its unmasked tokens
    (padded with zeroed masked tokens), cut into the shared PE-tile schedule
    (full tiles + packed tails), cast enc to bf16 and pre-transpose to
    [ti, i, p, n] so the device streams contiguous already-transposed
    tiles."""
    import ml_dtypes

    enc_output = np.asarray(enc_output, dtype=np.float32)
    decoder_hidden = np.asarray(decoder_hidden, dtype=np.float32)
    masks_b = np.asarray(masks).reshape(B, S).astype(bool)
    order, kps, kmins = plan_from_masks(masks_b)
    tiles, nslots, needs_mask, last_tile = make_schedule(kps, kmins)
    NTILE = len(tiles)

    # compacted enc/mask in tile layout, laid out per core
    enc_t = np.zeros((NCORES, NTILE, TT, E), dtype=np.float32)
    mask_t = np.ones((NCORES, NTILE * TT), dtype=np.uint8)
    for ti, segs in enumerate(tiles):
        for (pos, slot, off, w) in segs:
            for core in range(NCORES):
                g = int(order[pos * NCORES + core])
                idx = np.flatnonzero(~masks_b[g])[slot * TT: slot * TT + w]
                enc_t[core, ti, off:off + len(idx)] = enc_output[g, idx]
                mask_t[core, ti * TT + off: ti * TT + off + len(idx)] = 0
    # [TT, E] -> [P, NE, TT] (partition-major: contiguous per-partition DMA)
    encT = np.ascontiguousarray(
        enc_t.reshape(NCORES, NTILE, TT, NE, P).transpose(0, 1, 4, 3, 2)
    ).astype(ml_dtypes.bfloat16)

    shared = {
        "w_enc": np.asarray(W_enc, dtype=np.float32).astype(ml_dtypes.bfloat16),
        "b_enc": np.asarray(b_enc, dtype=np.float32).reshape(A),
        "w_dec": np.asarray(W_dec, dtype=np.float32).astype(ml_dtypes.bfloat16),
        "b_dec": np.asarray(b_dec, dtype=np.float32).reshape(A),
        "w_att": np.asarray(W_att, dtype=np.float32).reshape(A),
    }
    in_maps = []
    for c in range(NCORES):
        # decT[p, hc*BLOC + pos] = dec[order[pos*NCORES+c], hc*P + p]
        bidx = [int(order[pos * NCORES + c]) for pos in range(BLOC)]
        decT = np.ascontiguousarray(
            decoder_hidden[bidx].reshape(BLOC, HID // P, P)
            .transpose(2, 1, 0).reshape(P, (HID // P) * BLOC)
        ).astype(ml_dtypes.bfloat16)
        in_maps.append({
            "encT": encT[c],
            "decT": decT,
            "masks": mask_t[c],
            **shared,
        })
    return in_maps, order, kps, kmins


def kernel(enc_output, decoder_hidden, masks, W_enc, b_enc, W_dec, b_dec,
           W_att, b_att, **kwargs):
    # b_att shifts every score equally -> cancels in softmax; output does not
    # depend on it, so it is not shipped to the device.
    in_maps, order, kps, kmins = host_prep(enc_output, decoder_hidden, masks,
                                           W_enc, b_enc, W_dec, b_dec, W_att,
                                           b_att)
    res = run_bass_kernel_spmd(_get_nc(kps, kmins), in_maps,
                               core_ids=list(range(NCORES)))
    stacked = np.concatenate([res.results[c]["out"] for c in range(NCORES)],
                             axis=0)  # [core*BLOC + pos]
    out = np.empty_like(stacked)
    for pos in range(BLOC):
        for core in range(NCORES):
            out[int(order[pos * NCORES + core])] = stacked[core * BLOC + pos]
    return out


# revision 33
# speedup vs baseline: 1.0035x; 1.0035x over previous
"""Trainium2 Bass kernel for nn_Attn attention-context module.

Computation (per batch b):
    enc_att = enc @ W_enc + b_enc                      # [S, A]
    dec_att = dec @ W_dec + b_dec                      # [A]
    scores  = tanh(enc_att + dec_att) @ W_att + b_att  # [S]
    w       = softmax(mask(scores))                    # over S
    out     = sum_s w[s] * enc_att[s]                  # [A]

Strategy: data-parallel over batch across 8 NeuronCores (4 batches each),
weights replicated.

Masked tokens contribute exactly zero to the softmax numerator, denominator
and context (their score gets -32768 folded in, and exp underflows to +0), so
the host compacts each batch to its unmasked tokens, padded with zeroed,
masked-out tokens up to a global per-batch token count Kp (multiple of 64,
shared by all batches so the 8 cores run one instruction stream). With the
reference's ~50% mask density this roughly halves all device work. The
compacted enc is cast to bf16 and pre-transposed on the host so each core
streams contiguous, already-transposed bf16 tiles straight from HBM.

Device schedule: the token stream is cut into 512-wide PE tiles; the sub-512
tails of all batch positions are packed side by side into shared multi-segment
tiles so the PE never runs skinny matmuls (each tile = 32 dense N<=512 MMs).

Per tile:
  - PE computes enc_attT chunks [A-chunk(128), w tok] in PSUM (bf16 in,
    fp32 acc)
  - ACT applies tanh (bf16 out) with per-partition bias = dec_att + b_enc
    (+ b_dec), per segment; raw enc_att is copied to SBUF bf16 (ACT and DVE
    split the 4 chunks) for the context accumulation
  - scores: DVE premultiplies tanh by W_att per A-chunk and accumulates
    across the 4 chunks (tensor_scalar + 3 scalar_tensor_tensor, bf16), then
    ONE K=128 PE matmul with an all-ones lhsT reduces across partitions and
    broadcasts the score row to all 128 partitions of a PSUM tile; the mask
    is folded in as a -32768*mask K=1 matmul term (exp underflows to 0,
    killing masked + padding tokens)
  - softmax without max-subtraction (|scores| <= ||W_att||_1 ~ 51, exp can't
    overflow fp32; b_att cancels in the softmax so it is dropped); exp runs
    on the broadcast PSUM scores per segment, yielding bf16 numerators
    already replicated across partitions and per-partition denominators via
    accum_out (so no broadcast matmuls are needed anywhere downstream)
  - context accumulated per segment with fused DVE multiply+row-sum
    (scalar_tensor_tensor with accum_out, all-bf16 operands for 2x DVE);
    normalization and b_enc are applied once per batch (reciprocal is
    per-partition, again no broadcast needed)
"""

import os
import sys

import numpy as np

for _p in ("/opt/trn_rl_repo", "/root/.axon_site/_ro/trn_rl_repo"):
    if os.path.isdir(_p) and _p not in sys.path:
        sys.path.append(_p)

import concourse.bass as bass
import bass_rust
import concourse.mybir as mybir
from concourse import tile
from concourse.bass_utils import run_bass_kernel_spmd

P = 128
E = 1024          # 2*HIDDEN
A = 512           # ATT
HID = 512
S = 2048
B = 32
NCORES = 8
BLOC = B // NCORES           # 4 batches per core
TT = 512                     # tokens per PE tile
NE = E // P                  # 8 E-chunks
NA = A // P                  # 4 A-chunks

f32 = mybir.dt.float32
bf16 = mybir.dt.bfloat16
u8 = mybir.dt.uint8

_CACHE = {}


GRAN = 16  # padding granularity (matmul N is arbitrary)


def make_schedule(kps, kmins):
    """Cut the padded per-batch token streams into PE tiles.

    Full 512-wide tiles are single-segment; the sub-512 tails of all batch
    positions are packed side by side into shared multi-segment tiles.
    Returns (tiles, nslots, needs_mask, last_tile):
      tiles[ti]    = [(b, slot, off, w), ...]
      nslots[b]    = number of (b, slot) pairs for batch position b
      needs_mask[ti] = tile contains masked or padding tokens on some core
      last_tile[b] = index of the tile holding b's last slot
    """
    tiles, tails, nslots = [], [], []
    for b, kp in enumerate(kps):
        nf = kp // TT
        for t in range(nf):
            tiles.append([(b, t, 0, TT)])
        if kp % TT:
            tails.append((b, nf, kp % TT))
        nslots.append(nf + (1 if kp % TT else 0))
    cur, curw = [], 0
    for b, slot, w in tails:
        if curw + w > TT:
            tiles.append(cur)
            cur, curw = [], 0
        cur.append((b, slot, curw, w))
        curw += w
    if cur:
        tiles.append(cur)
    needs_mask = [
        any(slot * TT + w > kmins[b] for (b, slot, off, w) in segs)
        for segs in tiles
    ]
    last_tile = [max(ti for ti, segs in enumerate(tiles)
                     if any(s[0] == b for s in segs)) for b in range(BLOC)]
    return tiles, nslots, needs_mask, last_tile


def _split_multiwaits(nc):
    """This toolchain's walrus encodes at most 1 sync-wait per instruction
    (2 for EventSemaphore). Hoist extra waits onto pure-wait EventSemaphore
    instructions inserted immediately before the offender (same engine), which
    preserves semantics exactly."""
    n_split = 0
    uid = 0
    for fn in nc.m.functions:
        for blk in fn.blocks:
            new_insts = []
            for inst in blk.instructions:
                cap = 2 if type(inst).__name__ == "InstEventSemaphore" else 1
                si = inst.sync_info
                waits = list(si.on_wait) if si is not None and si.on_wait else []
                if len(waits) > cap:
                    extra, keep = waits[:-cap], waits[-cap:]
                    for i in range(0, len(extra), 2):
                        uid += 1
                        new_insts.append(bass_rust.InstEventSemaphore(
                            name=f"splitwait_{uid}_{inst.name}",
                            engine=inst.engine,
                            ins=[],
                            outs=[],
                            sync_info=bass_rust.SyncInfo(
                                on_wait=list(extra[i:i + 2]), on_update=[]),
                        ))
                        n_split += 1
                    si.on_wait = keep
                new_insts.append(inst)
            blk.instructions[:] = new_insts
    return n_split


def build(kps=None, kmins=None, encbufs=4, tebufs=4, eabufs=5,
          reps=1, probe=None, dmaq=1, scores_pe=False):
    if kps is None:
        kps = [S] * BLOC
    if kmins is None:
        kmins = [0] * BLOC
    tiles, nslots, needs_mask, last_tile = make_schedule(kps, kmins)
    NTILE = len(tiles)
    NSLOT = max(nslots)

    nc = bass.Bass("TRN2", debug=False)
    # host-compacted, pre-transposed bf16 enc in tile layout, partition-major
    # so a full tile is one fully-contiguous 8 KiB-per-partition DMA:
    # [ti, p, i, n] = enc_tile[ti, n, i*P+p]
    encT = nc.dram_tensor("encT", [NTILE, P, NE, TT], bf16,
                          kind="ExternalInput")
    # decoder_hidden, host-transposed: [p, hc*BLOC + b] = dec[b, hc*P + p]
    decT = nc.dram_tensor("decT", [P, (HID // P) * BLOC], bf16,
                          kind="ExternalInput")
    # compacted masks in tile layout, padded region = 1: [ti*TT + n]
    masks = nc.dram_tensor("masks", [NTILE * TT], u8, kind="ExternalInput")
    w_enc = nc.dram_tensor("w_enc", [E, A], bf16, kind="ExternalInput")
    b_enc = nc.dram_tensor("b_enc", [A], f32, kind="ExternalInput")
    w_dec = nc.dram_tensor("w_dec", [HID, A], bf16, kind="ExternalInput")
    b_dec = nc.dram_tensor("b_dec", [A], f32, kind="ExternalInput")
    w_att = nc.dram_tensor("w_att", [A], f32, kind="ExternalInput")
    out = nc.dram_tensor("out", [BLOC, A], f32, kind="ExternalOutput")

    Tanh = mybir.ActivationFunctionType.Tanh
    Exp = mybir.ActivationFunctionType.Exp
    Copy = mybir.ActivationFunctionType.Copy
    add = mybir.AluOpType.add
    mult = mybir.AluOpType.mult
    X = mybir.AxisListType.X

    with tile.TileContext(nc) as tc:
        with (
            tc.tile_pool(name="const", bufs=1) as cp,
            tc.tile_pool(name="encT", bufs=encbufs) as encp,
            tc.tile_pool(name="tanh", bufs=tebufs) as tanhp,
            tc.tile_pool(name="ea", bufs=eabufs) as eap,
            tc.tile_pool(name="wt", bufs=4) as wtp,
            tc.tile_pool(name="small", bufs=3) as smp,
            tc.tile_pool(name="attps", bufs=2, space="PSUM") as attp,
            tc.tile_pool(name="scps", bufs=2, space="PSUM") as scp,
        ):
            # ---------------- one-time prep ----------------
            # W_enc bf16: [e' part, (i, a)] for e = i*128 + e'
            wsb = cp.tile([P, NE * A], bf16, tag="wsb")
            nc.gpsimd.dma_start(
                wsb[:].rearrange("p (i a) -> p i a", i=NE),
                w_enc.ap().rearrange("(i p) a -> p i a", p=P))
            # W_dec bf16: [h' part, (i, a)] for h = i*128 + h'
            wdsb = cp.tile([P, (HID // P) * A], bf16, tag="wdsb")
            nc.scalar.dma_start(
                wdsb[:].rearrange("p (i a) -> p i a", i=HID // P),
                w_dec.ap().rearrange("(i p) a -> p i a", p=P))
            # W_att f32 column chunks [a' part, j] (per-partition STT scalar)
            wasf = cp.tile([P, NA], f32, tag="wasf")
            nc.scalar.dma_start(wasf[:], w_att.ap().rearrange("(j p) -> p j", p=P))
            # biases as column chunks [a' part, j]
            besb = cp.tile([P, NA], f32, tag="besb")
            nc.scalar.dma_start(besb[:], b_enc.ap().rearrange("(j p) -> p j", p=P))
            bdsb = cp.tile([P, NA], f32, tag="bdsb")
            nc.scalar.dma_start(bdsb[:], b_dec.ap().rearrange("(j p) -> p j", p=P))
            bbsb = cp.tile([P, NA], f32, tag="bbsb")
            nc.vector.tensor_tensor(bbsb[:], besb[:], bdsb[:], op=add)
            # decoder_hidden transposed [h' part, (hc, b)] (host-prepped)
            dhT = cp.tile([P, (HID // P) * BLOC], bf16, tag="dhT")
            nc.scalar.dma_start(dhT[:], decT.ap())
            # masks, whole core's worth: [1, NTILE*TT] u8 -> bf16
            msku = cp.tile([1, NTILE * TT], u8, tag="msku")
            nc.gpsimd.dma_start(msku[:], masks.ap()[None, :])
            mskf = cp.tile([1, NTILE * TT], bf16, tag="mskf")
            nc.vector.tensor_copy(mskf[:], msku[:])
            # all-ones lhsT: one K=128 matmul both reduces the premultiplied
            # tanh across partitions and broadcasts the scores to M=128
            ones128 = cp.tile([P, P], bf16, tag="ones128")
            nc.vector.memset(ones128[:], 1.0)
            # replicated W_att lhsT chunks (scores_pe variant): every output
            # column m of chunk j gets W_att[j*128+p], so the score matmul
            # reduces AND broadcasts without any DVE premultiply
            wrep = cp.tile([P, NA * P], bf16, tag="wrep")
            for j in range(NA):
                nc.vector.tensor_scalar(
                    out=wrep[:, j * P:(j + 1) * P], in0=ones128[:],
                    scalar1=wasf[:, j:j + 1], scalar2=None, op0=mult)
            # mask weight row for the -32768*mask K=1 broadcast matmul term
            m30row = cp.tile([1, P], bf16, tag="m30row")
            nc.vector.memset(m30row[:], -32768.0)

            # dec_attT + bias columns: bias_sb[a', j*BLOC + b]
            bias_sb = cp.tile([P, NA * BLOC], f32, tag="bias_sb")
            for j in range(NA):
                pd = scp.tile([P, BLOC], f32, tag="sc")
                for hc in range(HID // P):
                    nc.tensor.matmul(
                        pd[:],
                        lhsT=wdsb[:, hc * A + j * P: hc * A + (j + 1) * P],
                        rhs=dhT[:, hc * BLOC:(hc + 1) * BLOC],
                        start=(hc == 0), stop=(hc == HID // P - 1))
                nc.vector.tensor_scalar(
                    out=bias_sb[:, j * BLOC:(j + 1) * BLOC], in0=pd[:],
                    scalar1=bbsb[:, j:j + 1], scalar2=None, op0=add)

            # persistent accumulators (per-partition broadcast copies)
            ctxp = cp.tile([P, NA * BLOC * NSLOT], f32, tag="ctxp")
            ctxs = cp.tile([P, NA * BLOC], f32, tag="ctxs")
            densP = cp.tile([P, BLOC * NSLOT], f32, tag="densP")
            dentP = cp.tile([P, BLOC], f32, tag="dentP")
            recP = cp.tile([P, BLOC], f32, tag="recP")
            outsb = cp.tile([P, NA * BLOC], f32, tag="outsb")

            # ---------------- main loop ----------------
            # Two-stage deferred epilogue: at iteration t, the DVE premult of
            # tile t-1 is issued BEFORE tile t's main matmuls (so it executes
            # while the PE streams tile t), and the finish stage (reduce
            # matmul -> exp -> context) is issued after them (so its PE
            # matmul lands behind dense main work with its DVE input long
            # done). enc_att chunks j0/j1 are copied to SBUF bf16; j2/j3 stay
            # in PSUM and the context reads them there, saving two copies —
            # their banks (attB, 4 bufs) recycle two tiles later, after the
            # context STT has consumed them.
            premult_q, finish_q = [], []

            def premult_stage(ti, wt_, tanh_sb):
                if probe in ("dma", "mains", "tanhonly", "noscore") or scores_pe:
                    return None
                # wt = sum_j W_att_j * tanh_j   (bf16 chain on DVE)
                prev = wtp.tile([P, TT], bf16, tag="wt")
                nc.vector.tensor_scalar(
                    out=prev[:, :wt_], in0=tanh_sb[:, 0:wt_],
                    scalar1=wasf[:, 0:1], scalar2=None, op0=mult)
                for j in range(1, NA):
                    nxt = wtp.tile([P, TT], bf16, tag="wt")
                    nc.vector.scalar_tensor_tensor(
                        out=nxt[:, :wt_], in0=tanh_sb[:, j * wt_:(j + 1) * wt_],
                        scalar=wasf[:, j:j + 1], in1=prev[:, :wt_],
                        op0=mult, op1=add)
                    prev = nxt
                return prev

            def finish_stage(ti, segs, wt_, wt_tile, ea_sb, att23, tanh_sb):
                if probe in ("dma", "mains", "tanhonly", "noscore"):
                    return
                # broadcast scores [128, w]: reduce premult across partitions
                sc = scp.tile([P, TT], f32, tag="sc")
                nm = needs_mask[ti]
                if scores_pe:
                    for j in range(NA):
                        nc.tensor.matmul(
                            sc[:, :wt_], lhsT=wrep[:, j * P:(j + 1) * P],
                            rhs=tanh_sb[:, j * wt_:(j + 1) * wt_],
                            start=(j == 0), stop=(j == NA - 1 and not nm))
                else:
                    nc.tensor.matmul(sc[:, :wt_], lhsT=ones128[:],
                                     rhs=wt_tile[:, :wt_], start=True,
                                     stop=not nm)
                if nm:
                    nc.tensor.matmul(
                        sc[:, :wt_], lhsT=m30row[:],
                        rhs=mskf[0:1, ti * TT: ti * TT + wt_],
                        start=False, stop=True)
                # exp per segment (per-partition denominators via accum_out)
                p_sb = smp.tile([P, TT], bf16, tag="p_sb")
                for (b, slot, off, w) in segs:
                    bt = b * NSLOT + slot
                    nc.scalar.activation(
                        p_sb[:, off:off + w], sc[:, off:off + w], Exp,
                        accum_out=densP[:, bt:bt + 1])
                if probe == "noctx":
                    return
                # fused context accumulation per (A-chunk, segment):
                # accum_out = sum_tok(p * enc_att) per partition.
                # j2/j3 first — they read (and free) the PSUM att banks.
                waste = smp.tile([P, TT], bf16, tag="waste")
                for j in (2, 3, 0, 1):
                    for (b, slot, off, w) in segs:
                        src = (att23[j - 2][:, off:off + w] if j >= 2 else
                               ea_sb[:, j * TT + off: j * TT + off + w])
                        nc.vector.scalar_tensor_tensor(
                            out=waste[:, off:off + w],
                            in0=p_sb[:, off:off + w], scalar=1.0,
                            in1=src, op0=mult, op1=mult,
                            accum_out=ctxp[:, (j * BLOC + b) * NSLOT + slot:
                                           (j * BLOC + b) * NSLOT + slot + 1])

            def batch_epilogue(b):
                if probe in ("dma", "mains", "tanhonly", "noscore", "noctx"):
                    return
                ns = nslots[b]
                # out[b] = ctx/den + b_enc (everything already per-partition)
                nc.vector.reduce_sum(
                    dentP[:, b:b + 1], densP[:, b * NSLOT:b * NSLOT + ns],
                    axis=X)
                nc.vector.reciprocal(recP[:, b:b + 1], dentP[:, b:b + 1])
                nc.vector.reduce_sum(
                    ctxs[:, b * NA:(b + 1) * NA],
                    ctxp[:].rearrange("p (j bb t) -> p j bb t", j=NA, bb=BLOC)
                    [:, :, b, :ns], axis=X)
                nc.vector.scalar_tensor_tensor(
                    out=outsb[:, b * NA:(b + 1) * NA],
                    in0=ctxs[:, b * NA:(b + 1) * NA],
                    scalar=recP[:, b:b + 1], in1=besb[:], op0=mult, op1=add)
                nc.scalar.dma_start(
                    out.ap()[b].rearrange("(j p) -> p j", p=P),
                    outsb[:, b * NA:(b + 1) * NA])

            for ti, segs in [(t_, s_) for _ in range(reps)
                             for t_, s_ in enumerate(tiles)]:
                wt_ = sum(s[3] for s in segs)
                if premult_q:
                    premult_q.pop(0)()
                # load pre-transposed tile: encTt[e', (i, n)], n < w
                encTt = encp.tile([P, NE * TT], bf16, tag="encT")
                dmae = nc.sync if (ti % 2 == 0 or dmaq == 1) else nc.scalar
                if wt_ == TT:
                    dmae.dma_start(
                        encTt[:], encT.ap()[ti].rearrange("p i n -> p (i n)"))
                else:
                    dmae.dma_start(
                        encTt[:].rearrange("p (i n) -> p i n", i=NE)
                        [:, :, :wt_],
                        encT.ap()[ti][:, :, :wt_])

                tanh_sb = tanhp.tile([P, NA * TT], bf16, tag="tanh")
                ea_sb = eap.tile([P, 2 * TT], bf16, tag="ea")
                att23 = []
                for j in range(NA if probe != "dma" else 0):
                    if j == 2 and finish_q and wt_ >= 384:
                        # previous tile's finish stage lands mid-tile: its
                        # reduce matmul slots between dense j1/j2 groups with
                        # its premult input long done, exp fires mid-tile,
                        # and the attB banks recycle with margin to spare
                        while finish_q:
                            finish_q.pop(0)()
                    if j < 2:
                        att = attp.tile([P, TT], f32, tag="attA")
                    else:
                        att = attp.tile([P, TT], f32, tag="attB", bufs=4)
                        att23.append(att)
                    for i in range(NE):
                        nc.tensor.matmul(
                            att[:, :wt_],
                            lhsT=wsb[:, i * A + j * P: i * A + (j + 1) * P],
                            rhs=encTt[:, i * TT:i * TT + wt_],
                            start=(i == 0), stop=(i == NE - 1))
                    if probe == "mains":
                        continue
                    for (b, slot, off, w) in segs:
                        nc.scalar.activation(
                            tanh_sb[:, j * wt_ + off: j * wt_ + off + w],
                            att[:, off:off + w], Tanh,
                            bias=bias_sb[:, j * BLOC + b: j * BLOC + b + 1])
                    # copy raw enc_att j0/j1 out of PSUM (ACT); j2/j3 stay
                    # in PSUM for the context and free their banks there.
                    if probe == "tanhonly" or j >= 2:
                        continue
                    nc.scalar.activation(
                        ea_sb[:, j * TT:j * TT + wt_], att[:, :wt_], Copy)

                while finish_q:  # short tiles: pop at end instead
                    finish_q.pop(0)()
                wt_holder = []
                premult_q.append(
                    lambda ti=ti, wt_=wt_, ts=tanh_sb, h=wt_holder:
                    h.append(premult_stage(ti, wt_, ts)))
                finish_q.append(
                    lambda ti=ti, segs=segs, wt_=wt_, h=wt_holder, a=ea_sb,
                    a23=att23, ts=tanh_sb:
                    finish_stage(ti, segs, wt_, h[0], a, a23, ts))
                for b in range(BLOC):
                    if last_tile[b] == ti:
                        finish_q.append(lambda b=b: batch_epilogue(b))
            while premult_q:
                premult_q.pop(0)()
            while finish_q:
                finish_q.pop(0)()

    n = _split_multiwaits(nc)
    if os.environ.get("KERNEL_DEBUG"):
        print(f"[kernel] split {n} extra waits", file=sys.stderr)
    return nc


def _get_nc(kps, kmins):
    key = (tuple(kps), tuple(kmins))
    if key not in _CACHE:
        _CACHE[key] = build(kps=list(kps), kmins=list(kmins))
    return _CACHE[key]


def plan_from_masks(masks):
    """Load-balanced batch -> (core, position) assignment. Batches are sorted
    by unmasked count (descending) and dealt position-major, so each of the
    BLOC schedule positions covers 8 similarly-sized batches and its padded
    width is the max over those 8. Returns (order, kps, kmins) where
    order[pos * NCORES + core] = original batch index."""
    m = np.asarray(masks).reshape(B, S).astype(bool)
    k = (~m).sum(axis=1).astype(int)
    order = np.argsort(-k, kind="stable")
    kps, kmins = [], []
    for pos in range(BLOC):
        grp = order[pos * NCORES:(pos + 1) * NCORES]
        kmax = max(max(int(k[g]) for g in grp), GRAN)
        kps.append(-(-kmax // GRAN) * GRAN)
        kmins.append(min(int(k[g]) for g in grp))
    return order, kps, kmins


def host_prep(enc_output, decoder_hidden, masks, W_enc, b_enc, W_dec, b_dec,
              W_att, b_att=None, **kwargs):
    """Shard + lay out inputs for the 8 cores: assign batches to (core,
    position) by unmasked count, compact each batch to its unmasked tokens
    (padded with zeroed masked tokens), cut into the shared PE-tile schedule
    (full tiles + packed tails), cast enc to bf16 and pre-transpose to
    [ti, i, p, n] so the device streams contiguous already-transposed
    tiles."""
    import ml_dtypes

    enc_output = np.asarray(enc_output, dtype=np.float32)
    decoder_hidden = np.asarray(decoder_hidden, dtype=np.float32)
    masks_b = np.asarray(masks).reshape(B, S).astype(bool)
    order, kps, kmins = plan_from_masks(masks_b)
    tiles, nslots, needs_mask, last_tile = make_schedule(kps, kmins)
    NTILE = len(tiles)

    # compacted enc/mask in tile layout, laid out per core
    enc_t = np.zeros((NCORES, NTILE, TT, E), dtype=np.float32)
    mask_t = np.ones((NCORES, NTILE * TT), dtype=np.uint8)
    for ti, segs in enumerate(tiles):
        for (pos, slot, off, w) in segs:
            for core in range(NCORES):
                g = int(order[pos * NCORES + core])
                idx = np.flatnonzero(~masks_b[g])[slot * TT: slot * TT + w]
                enc_t[core, ti, off:off + len(idx)] = enc_output[g, idx]
                mask_t[core, ti * TT + off: ti * TT + off + len(idx)] = 0
    # [TT, E] -> [P, NE, TT] (partition-major: contiguous per-partition DMA)
    encT = np.ascontiguousarray(
        enc_t.reshape(NCORES, NTILE, TT, NE, P).transpose(0, 1, 4, 3, 2)
    ).astype(ml_dtypes.bfloat16)

    shared = {
        "w_enc": np.asarray(W_enc, dtype=np.float32).astype(ml_dtypes.bfloat16),
        "b_enc": np.asarray(b_enc, dtype=np.float32).reshape(A),
        "w_dec": np.asarray(W_dec, dtype=np.float32).astype(ml_dtypes.bfloat16),
        "b_dec": np.asarray(b_dec, dtype=np.float32).reshape(A),
        "w_att": np.asarray(W_att, dtype=np.float32).reshape(A),
    }
    in_maps = []
    for c in range(NCORES):
        # decT[p, hc*BLOC + pos] = dec[order[pos*NCORES+c], hc*P + p]
        bidx = [int(order[pos * NCORES + c]) for pos in range(BLOC)]
        decT = np.ascontiguousarray(
            decoder_hidden[bidx].reshape(BLOC, HID // P, P)
            .transpose(2, 1, 0).reshape(P, (HID // P) * BLOC)
        ).astype(ml_dtypes.bfloat16)
        in_maps.append({
            "encT": encT[c],
            "decT": decT,
            "masks": mask_t[c],
            **shared,
        })
    return in_maps, order, kps, kmins


def kernel(enc_output, decoder_hidden, masks, W_enc, b_enc, W_dec, b_dec,
           W_att, b_att, **kwargs):
    # b_att shifts every score equally -> cancels in softmax; output does not
    # depend on it, so it is not shipped to the device.
    in_maps, order, kps, kmins = host_prep(enc_output, decoder_hidden, masks,
                                           W_enc, b_enc, W_dec, b_dec, W_att,
                                           b_att)
    res = run_bass_kernel_spmd(_get_nc(kps, kmins), in_maps,
                               core_ids=list(range(NCORES)))
    stacked = np.concatenate([res.results[c]["out"] for c in range(NCORES)],
                             axis=0)  # [core*BLOC + pos]
    out = np.empty_like(stacked)
    for pos in range(BLOC):
        for core in range(NCORES):
            out[int(order[pos * NCORES + core])] = stacked[core * BLOC + pos]
    return out


# revision 35
# speedup vs baseline: 1.0101x; 1.0065x over previous
"""Trainium2 Bass kernel for nn_Attn attention-context module.

Computation (per batch b):
    enc_att = enc @ W_enc + b_enc                      # [S, A]
    dec_att = dec @ W_dec + b_dec                      # [A]
    scores  = tanh(enc_att + dec_att) @ W_att + b_att  # [S]
    w       = softmax(mask(scores))                    # over S
    out     = sum_s w[s] * enc_att[s]                  # [A]

Strategy: data-parallel over batch across 8 NeuronCores (4 batches each),
weights replicated.

Masked tokens contribute exactly zero to the softmax numerator, denominator
and context (their score gets -32768 folded in, and exp underflows to +0), so
the host compacts each batch to its unmasked tokens, padded with zeroed,
masked-out tokens up to a global per-batch token count Kp (multiple of 64,
shared by all batches so the 8 cores run one instruction stream). With the
reference's ~50% mask density this roughly halves all device work. The
compacted enc is cast to bf16 and pre-transposed on the host so each core
streams contiguous, already-transposed bf16 tiles straight from HBM.

Device schedule: the token stream is cut into 512-wide PE tiles; the sub-512
tails of all batch positions are packed side by side into shared multi-segment
tiles so the PE never runs skinny matmuls (each tile = 32 dense N<=512 MMs).

Per tile:
  - PE computes enc_attT chunks [A-chunk(128), w tok] in PSUM (bf16 in,
    fp32 acc)
  - ACT applies tanh (bf16 out) with per-partition bias = dec_att + b_enc
    (+ b_dec), per segment; raw enc_att is copied to SBUF bf16 (ACT and DVE
    split the 4 chunks) for the context accumulation
  - scores: DVE premultiplies tanh by W_att per A-chunk and accumulates
    across the 4 chunks (tensor_scalar + 3 scalar_tensor_tensor, bf16), then
    ONE K=128 PE matmul with an all-ones lhsT reduces across partitions and
    broadcasts the score row to all 128 partitions of a PSUM tile; the mask
    is folded in as a -32768*mask K=1 matmul term (exp underflows to 0,
    killing masked + padding tokens)
  - softmax without max-subtraction (|scores| <= ||W_att||_1 ~ 51, exp can't
    overflow fp32; b_att cancels in the softmax so it is dropped); exp runs
    on the broadcast PSUM scores per segment, yielding bf16 numerators
    already replicated across partitions and per-partition denominators via
    accum_out (so no broadcast matmuls are needed anywhere downstream)
  - context accumulated per segment with fused DVE multiply+row-sum
    (scalar_tensor_tensor with accum_out, all-bf16 operands for 2x DVE);
    normalization and b_enc are applied once per batch (reciprocal is
    per-partition, again no broadcast needed)
"""

import os
import sys

import numpy as np

for _p in ("/opt/trn_rl_repo", "/root/.axon_site/_ro/trn_rl_repo"):
    if os.path.isdir(_p) and _p not in sys.path:
        sys.path.append(_p)

import concourse.bass as bass
import bass_rust
import concourse.mybir as mybir
from concourse import tile
from concourse.bass_utils import run_bass_kernel_spmd

P = 128
E = 1024          # 2*HIDDEN
A = 512           # ATT
HID = 512
S = 2048
B = 32
NCORES = 8
BLOC = B // NCORES           # 4 batches per core
TT = 512                     # tokens per PE tile
NE = E // P                  # 8 E-chunks
NA = A // P                  # 4 A-chunks

f32 = mybir.dt.float32
bf16 = mybir.dt.bfloat16
u8 = mybir.dt.uint8

_CACHE = {}


GRAN = 16  # padding granularity (matmul N is arbitrary)


def make_schedule(kps, kmins):
    """Cut the padded per-batch token streams into PE tiles.

    Full 512-wide tiles are single-segment; the sub-512 tails of all batch
    positions are packed side by side into shared multi-segment tiles.
    Returns (tiles, nslots, needs_mask, last_tile):
      tiles[ti]    = [(b, slot, off, w), ...]
      nslots[b]    = number of (b, slot) pairs for batch position b
      needs_mask[ti] = tile contains masked or padding tokens on some core
      last_tile[b] = index of the tile holding b's last slot
    """
    tiles, tails, nslots = [], [], []
    for b, kp in enumerate(kps):
        nf = kp // TT
        for t in range(nf):
            tiles.append([(b, t, 0, TT)])
        if kp % TT:
            tails.append((b, nf, kp % TT))
        nslots.append(nf + (1 if kp % TT else 0))
    cur, curw = [], 0
    for b, slot, w in tails:
        if curw + w > TT:
            tiles.append(cur)
            cur, curw = [], 0
        cur.append((b, slot, curw, w))
        curw += w
    if cur:
        tiles.append(cur)
    needs_mask = [
        any(slot * TT + w > kmins[b] for (b, slot, off, w) in segs)
        for segs in tiles
    ]
    last_tile = [max(ti for ti, segs in enumerate(tiles)
                     if any(s[0] == b for s in segs)) for b in range(BLOC)]
    return tiles, nslots, needs_mask, last_tile


def _split_multiwaits(nc):
    """This toolchain's walrus encodes at most 1 sync-wait per instruction
    (2 for EventSemaphore). Hoist extra waits onto pure-wait EventSemaphore
    instructions inserted immediately before the offender (same engine), which
    preserves semantics exactly."""
    n_split = 0
    uid = 0
    for fn in nc.m.functions:
        for blk in fn.blocks:
            new_insts = []
            for inst in blk.instructions:
                cap = 2 if type(inst).__name__ == "InstEventSemaphore" else 1
                si = inst.sync_info
                waits = list(si.on_wait) if si is not None and si.on_wait else []
                if len(waits) > cap:
                    extra, keep = waits[:-cap], waits[-cap:]
                    for i in range(0, len(extra), 2):
                        uid += 1
                        new_insts.append(bass_rust.InstEventSemaphore(
                            name=f"splitwait_{uid}_{inst.name}",
                            engine=inst.engine,
                            ins=[],
                            outs=[],
                            sync_info=bass_rust.SyncInfo(
                                on_wait=list(extra[i:i + 2]), on_update=[]),
                        ))
                        n_split += 1
                    si.on_wait = keep
                new_insts.append(inst)
            blk.instructions[:] = new_insts
    return n_split


def build(kps=None, kmins=None, encbufs=6, tebufs=5, eabufs=6,
          reps=1, probe=None, dmaq=2, scores_pe=False):
    if kps is None:
        kps = [S] * BLOC
    if kmins is None:
        kmins = [0] * BLOC
    tiles, nslots, needs_mask, last_tile = make_schedule(kps, kmins)
    NTILE = len(tiles)
    NSLOT = max(nslots)

    nc = bass.Bass("TRN2", debug=False)
    # host-compacted, pre-transposed bf16 enc in tile layout, partition-major
    # so a full tile is one fully-contiguous 8 KiB-per-partition DMA:
    # [ti, p, i, n] = enc_tile[ti, n, i*P+p]
    encT = nc.dram_tensor("encT", [NTILE, P, NE, TT], bf16,
                          kind="ExternalInput")
    # decoder_hidden, host-transposed: [p, hc*BLOC + b] = dec[b, hc*P + p]
    decT = nc.dram_tensor("decT", [P, (HID // P) * BLOC], bf16,
                          kind="ExternalInput")
    # compacted masks in tile layout, padded region = 1: [ti*TT + n]
    masks = nc.dram_tensor("masks", [NTILE * TT], u8, kind="ExternalInput")
    w_enc = nc.dram_tensor("w_enc", [E, A], bf16, kind="ExternalInput")
    b_enc = nc.dram_tensor("b_enc", [A], f32, kind="ExternalInput")
    w_dec = nc.dram_tensor("w_dec", [HID, A], bf16, kind="ExternalInput")
    b_dec = nc.dram_tensor("b_dec", [A], f32, kind="ExternalInput")
    w_att = nc.dram_tensor("w_att", [A], f32, kind="ExternalInput")
    out = nc.dram_tensor("out", [BLOC, A], f32, kind="ExternalOutput")

    Tanh = mybir.ActivationFunctionType.Tanh
    Exp = mybir.ActivationFunctionType.Exp
    Copy = mybir.ActivationFunctionType.Copy
    add = mybir.AluOpType.add
    mult = mybir.AluOpType.mult
    X = mybir.AxisListType.X

    with tile.TileContext(nc) as tc:
        with (
            tc.tile_pool(name="const", bufs=1) as cp,
            tc.tile_pool(name="encT", bufs=encbufs) as encp,
            tc.tile_pool(name="tanh", bufs=tebufs) as tanhp,
            tc.tile_pool(name="ea", bufs=eabufs) as eap,
            tc.tile_pool(name="wt", bufs=4) as wtp,
            tc.tile_pool(name="small", bufs=3) as smp,
            tc.tile_pool(name="attps", bufs=2, space="PSUM") as attp,
            tc.tile_pool(name="scps", bufs=2, space="PSUM") as scp,
        ):
            # ---------------- one-time prep ----------------
            # W_enc bf16: [e' part, (i, a)] for e = i*128 + e'
            wsb = cp.tile([P, NE * A], bf16, tag="wsb")
            nc.gpsimd.dma_start(
                wsb[:].rearrange("p (i a) -> p i a", i=NE),
                w_enc.ap().rearrange("(i p) a -> p i a", p=P))
            # W_dec bf16: [h' part, (i, a)] for h = i*128 + h'
            wdsb = cp.tile([P, (HID // P) * A], bf16, tag="wdsb")
            nc.scalar.dma_start(
                wdsb[:].rearrange("p (i a) -> p i a", i=HID // P),
                w_dec.ap().rearrange("(i p) a -> p i a", p=P))
            # W_att f32 column chunks [a' part, j] (per-partition STT scalar)
            wasf = cp.tile([P, NA], f32, tag="wasf")
            nc.scalar.dma_start(wasf[:], w_att.ap().rearrange("(j p) -> p j", p=P))
            # biases as column chunks [a' part, j]
            besb = cp.tile([P, NA], f32, tag="besb")
            nc.scalar.dma_start(besb[:], b_enc.ap().rearrange("(j p) -> p j", p=P))
            bdsb = cp.tile([P, NA], f32, tag="bdsb")
            nc.scalar.dma_start(bdsb[:], b_dec.ap().rearrange("(j p) -> p j", p=P))
            bbsb = cp.tile([P, NA], f32, tag="bbsb")
            nc.vector.tensor_tensor(bbsb[:], besb[:], bdsb[:], op=add)
            # decoder_hidden transposed [h' part, (hc, b)] (host-prepped)
            dhT = cp.tile([P, (HID // P) * BLOC], bf16, tag="dhT")
            nc.scalar.dma_start(dhT[:], decT.ap())
            # masks, whole core's worth: [1, NTILE*TT] u8 -> bf16
            msku = cp.tile([1, NTILE * TT], u8, tag="msku")
            nc.gpsimd.dma_start(msku[:], masks.ap()[None, :])
            mskf = cp.tile([1, NTILE * TT], bf16, tag="mskf")
            nc.vector.tensor_copy(mskf[:], msku[:])
            # all-ones lhsT: one K=128 matmul both reduces the premultiplied
            # tanh across partitions and broadcasts the scores to M=128
            ones128 = cp.tile([P, P], bf16, tag="ones128")
            nc.vector.memset(ones128[:], 1.0)
            # replicated W_att lhsT chunks (scores_pe variant): every output
            # column m of chunk j gets W_att[j*128+p], so the score matmul
            # reduces AND broadcasts without any DVE premultiply
            wrep = cp.tile([P, NA * P], bf16, tag="wrep")
            for j in range(NA):
                nc.vector.tensor_scalar(
                    out=wrep[:, j * P:(j + 1) * P], in0=ones128[:],
                    scalar1=wasf[:, j:j + 1], scalar2=None, op0=mult)
            # mask weight row for the -32768*mask K=1 broadcast matmul term
            m30row = cp.tile([1, P], bf16, tag="m30row")
            nc.vector.memset(m30row[:], -32768.0)

            # dec_attT + bias columns: bias_sb[a', j*BLOC + b]
            bias_sb = cp.tile([P, NA * BLOC], f32, tag="bias_sb")
            for j in range(NA):
                pd = scp.tile([P, BLOC], f32, tag="sc")
                for hc in range(HID // P):
                    nc.tensor.matmul(
                        pd[:],
                        lhsT=wdsb[:, hc * A + j * P: hc * A + (j + 1) * P],
                        rhs=dhT[:, hc * BLOC:(hc + 1) * BLOC],
                        start=(hc == 0), stop=(hc == HID // P - 1))
                nc.vector.tensor_scalar(
                    out=bias_sb[:, j * BLOC:(j + 1) * BLOC], in0=pd[:],
                    scalar1=bbsb[:, j:j + 1], scalar2=None, op0=add)

            # persistent accumulators (per-partition broadcast copies)
            ctxp = cp.tile([P, NA * BLOC * NSLOT], f32, tag="ctxp")
            ctxs = cp.tile([P, NA * BLOC], f32, tag="ctxs")
            densP = cp.tile([P, BLOC * NSLOT], f32, tag="densP")
            dentP = cp.tile([P, BLOC], f32, tag="dentP")
            recP = cp.tile([P, BLOC], f32, tag="recP")
            outsb = cp.tile([P, NA * BLOC], f32, tag="outsb")

            # ---------------- main loop ----------------
            # Two-stage deferred epilogue: at iteration t, the DVE premult of
            # tile t-1 is issued BEFORE tile t's main matmuls (so it executes
            # while the PE streams tile t), and the finish stage (reduce
            # matmul -> exp -> context) is issued after them (so its PE
            # matmul lands behind dense main work with its DVE input long
            # done). enc_att chunks j0/j1 are copied to SBUF bf16; j2/j3 stay
            # in PSUM and the context reads them there, saving two copies —
            # their banks (attB, 4 bufs) recycle two tiles later, after the
            # context STT has consumed them.
            premult_q, finish_q = [], []

            def premult_stage(ti, wt_, tanh_sb):
                if probe in ("dma", "mains", "tanhonly", "noscore") or scores_pe:
                    return None
                # wt = sum_j W_att_j * tanh_j   (bf16 chain on DVE)
                prev = wtp.tile([P, TT], bf16, tag="wt")
                nc.vector.tensor_scalar(
                    out=prev[:, :wt_], in0=tanh_sb[:, 0:wt_],
                    scalar1=wasf[:, 0:1], scalar2=None, op0=mult)
                for j in range(1, NA):
                    nxt = wtp.tile([P, TT], bf16, tag="wt")
                    nc.vector.scalar_tensor_tensor(
                        out=nxt[:, :wt_], in0=tanh_sb[:, j * wt_:(j + 1) * wt_],
                        scalar=wasf[:, j:j + 1], in1=prev[:, :wt_],
                        op0=mult, op1=add)
                    prev = nxt
                return prev

            def finish_stage(ti, segs, wt_, wt_tile, ea_sb, att23, tanh_sb):
                if probe in ("dma", "mains", "tanhonly", "noscore"):
                    return
                # broadcast scores [128, w]: reduce premult across partitions
                sc = scp.tile([P, TT], f32, tag="sc")
                nm = needs_mask[ti]
                if scores_pe:
                    for j in range(NA):
                        nc.tensor.matmul(
                            sc[:, :wt_], lhsT=wrep[:, j * P:(j + 1) * P],
                            rhs=tanh_sb[:, j * wt_:(j + 1) * wt_],
                            start=(j == 0), stop=(j == NA - 1 and not nm))
                else:
                    nc.tensor.matmul(sc[:, :wt_], lhsT=ones128[:],
                                     rhs=wt_tile[:, :wt_], start=True,
                                     stop=not nm)
                if nm:
                    nc.tensor.matmul(
                        sc[:, :wt_], lhsT=m30row[:],
                        rhs=mskf[0:1, ti * TT: ti * TT + wt_],
                        start=False, stop=True)
                # exp per segment (per-partition denominators via accum_out)
                p_sb = smp.tile([P, TT], bf16, tag="p_sb")
                for (b, slot, off, w) in segs:
                    bt = b * NSLOT + slot
                    nc.scalar.activation(
                        p_sb[:, off:off + w], sc[:, off:off + w], Exp,
                        accum_out=densP[:, bt:bt + 1])
                if probe == "noctx":
                    return
                # fused context accumulation per (A-chunk, segment):
                # accum_out = sum_tok(p * enc_att) per partition.
                # j2/j3 first — they read (and free) the PSUM att banks.
                waste = smp.tile([P, TT], bf16, tag="waste")
                for j in (2, 3, 0, 1):
                    for (b, slot, off, w) in segs:
                        src = (att23[j - 2][:, off:off + w] if j >= 2 else
                               ea_sb[:, j * TT + off: j * TT + off + w])
                        nc.vector.scalar_tensor_tensor(
                            out=waste[:, off:off + w],
                            in0=p_sb[:, off:off + w], scalar=1.0,
                            in1=src, op0=mult, op1=mult,
                            accum_out=ctxp[:, (j * BLOC + b) * NSLOT + slot:
                                           (j * BLOC + b) * NSLOT + slot + 1])

            def batch_epilogue(b):
                if probe in ("dma", "mains", "tanhonly", "noscore", "noctx"):
                    return
                ns = nslots[b]
                # out[b] = ctx/den + b_enc (everything already per-partition)
                nc.vector.reduce_sum(
                    dentP[:, b:b + 1], densP[:, b * NSLOT:b * NSLOT + ns],
                    axis=X)
                nc.vector.reciprocal(recP[:, b:b + 1], dentP[:, b:b + 1])
                nc.vector.reduce_sum(
                    ctxs[:, b * NA:(b + 1) * NA],
                    ctxp[:].rearrange("p (j bb t) -> p j bb t", j=NA, bb=BLOC)
                    [:, :, b, :ns], axis=X)
                nc.vector.scalar_tensor_tensor(
                    out=outsb[:, b * NA:(b + 1) * NA],
                    in0=ctxs[:, b * NA:(b + 1) * NA],
                    scalar=recP[:, b:b + 1], in1=besb[:], op0=mult, op1=add)
                nc.scalar.dma_start(
                    out.ap()[b].rearrange("(j p) -> p j", p=P),
                    outsb[:, b * NA:(b + 1) * NA])

            for ti, segs in [(t_, s_) for _ in range(reps)
                             for t_, s_ in enumerate(tiles)]:
                wt_ = sum(s[3] for s in segs)
                if premult_q:
                    premult_q.pop(0)()
                # load pre-transposed tile: encTt[e', (i, n)], n < w
                encTt = encp.tile([P, NE * TT], bf16, tag="encT")
                dmae = nc.sync if (ti % 2 == 0 or dmaq == 1) else nc.scalar
                if wt_ == TT:
                    dmae.dma_start(
                        encTt[:], encT.ap()[ti].rearrange("p i n -> p (i n)"))
                else:
                    dmae.dma_start(
                        encTt[:].rearrange("p (i n) -> p i n", i=NE)
                        [:, :, :wt_],
                        encT.ap()[ti][:, :, :wt_])

                tanh_sb = tanhp.tile([P, NA * TT], bf16, tag="tanh")
                ea_sb = eap.tile([P, 2 * TT], bf16, tag="ea")
                att23 = []
                for j in range(NA if probe != "dma" else 0):
                    if j == 2 and finish_q and wt_ >= 384:
                        # previous tile's finish stage lands mid-tile: its
                        # reduce matmul slots between dense j1/j2 groups with
                        # its premult input long done, exp fires mid-tile,
                        # and the attB banks recycle with margin to spare
                        while finish_q:
                            finish_q.pop(0)()
                    if j < 2:
                        att = attp.tile([P, TT], f32, tag="attA")
                    else:
                        att = attp.tile([P, TT], f32, tag="attB", bufs=4)
                        att23.append(att)
                    for i in range(NE):
                        nc.tensor.matmul(
                            att[:, :wt_],
                            lhsT=wsb[:, i * A + j * P: i * A + (j + 1) * P],
                            rhs=encTt[:, i * TT:i * TT + wt_],
                            start=(i == 0), stop=(i == NE - 1))
                    if probe == "mains":
                        continue
                    for (b, slot, off, w) in segs:
                        nc.scalar.activation(
                            tanh_sb[:, j * wt_ + off: j * wt_ + off + w],
                            att[:, off:off + w], Tanh,
                            bias=bias_sb[:, j * BLOC + b: j * BLOC + b + 1])
                    # copy raw enc_att j0/j1 out of PSUM (ACT); j2/j3 stay
                    # in PSUM for the context and free their banks there.
                    if probe == "tanhonly" or j >= 2:
                        continue
                    nc.scalar.activation(
                        ea_sb[:, j * TT:j * TT + wt_], att[:, :wt_], Copy)

                while finish_q:  # short tiles: pop at end instead
                    finish_q.pop(0)()
                wt_holder = []
                premult_q.append(
                    lambda ti=ti, wt_=wt_, ts=tanh_sb, h=wt_holder:
                    h.append(premult_stage(ti, wt_, ts)))
                finish_q.append(
                    lambda ti=ti, segs=segs, wt_=wt_, h=wt_holder, a=ea_sb,
                    a23=att23, ts=tanh_sb:
                    finish_stage(ti, segs, wt_, h[0], a, a23, ts))
                for b in range(BLOC):
                    if last_tile[b] == ti:
                        finish_q.append(lambda b=b: batch_epilogue(b))
            while premult_q:
                premult_q.pop(0)()
            while finish_q:
                finish_q.pop(0)()

    n = _split_multiwaits(nc)
    if os.environ.get("KERNEL_DEBUG"):
        print(f"[kernel] split {n} extra waits", file=sys.stderr)
    return nc


def _get_nc(kps, kmins):
    key = (tuple(kps), tuple(kmins))
    if key not in _CACHE:
        _CACHE[key] = build(kps=list(kps), kmins=list(kmins))
    return _CACHE[key]


def plan_from_masks(masks):
    """Load-balanced batch -> (core, position) assignment. Batches are sorted
    by unmasked count (descending) and dealt position-major, so each of the
    BLOC schedule positions covers 8 similarly-sized batches and its padded
    width is the max over those 8. Returns (order, kps, kmins) where
    order[pos * NCORES + core] = original batch index."""
    m = np.asarray(masks).reshape(B, S).astype(bool)
    k = (~m).sum(axis=1).astype(int)
    order = np.argsort(-k, kind="stable")
    kps, kmins = [], []
    for pos in range(BLOC):
        grp = order[pos * NCORES:(pos + 1) * NCORES]
        kmax = max(max(int(k[g]) for g in grp), GRAN)
        kps.append(-(-kmax // GRAN) * GRAN)
        kmins.append(min(int(k[g]) for g in grp))
    return order, kps, kmins


def host_prep(enc_output, decoder_hidden, masks, W_enc, b_enc, W_dec, b_dec,
              W_att, b_att=None, **kwargs):
    """Shard + lay out inputs for the 8 cores: assign batches to (core,
    position) by unmasked count, compact each batch to its unmasked tokens
    (padded with zeroed masked tokens), cut into the shared PE-tile schedule
    (full tiles + packed tails), cast enc to bf16 and pre-transpose to
    [ti, i, p, n] so the device streams contiguous already-transposed
    tiles."""
    import ml_dtypes

    enc_output = np.asarray(enc_output, dtype=np.float32)
    decoder_hidden = np.asarray(decoder_hidden, dtype=np.float32)
    masks_b = np.asarray(masks).reshape(B, S).astype(bool)
    order, kps, kmins = plan_from_masks(masks_b)
    tiles, nslots, needs_mask, last_tile = make_schedule(kps, kmins)
    NTILE = len(tiles)

    # compacted enc/mask in tile layout, laid out per core
    enc_t = np.zeros((NCORES, NTILE, TT, E), dtype=np.float32)
    mask_t = np.ones((NCORES, NTILE * TT), dtype=np.uint8)
    for ti, segs in enumerate(tiles):
        for (pos, slot, off, w) in segs:
            for core in range(NCORES):
                g = int(order[pos * NCORES + core])
                idx = np.flatnonzero(~masks_b[g])[slot * TT: slot * TT + w]
                enc_t[core, ti, off:off + len(idx)] = enc_output[g, idx]
                mask_t[core, ti * TT + off: ti * TT + off + len(idx)] = 0
    # [TT, E] -> [P, NE, TT] (partition-major: contiguous per-partition DMA)
    encT = np.ascontiguousarray(
        enc_t.reshape(NCORES, NTILE, TT, NE, P).transpose(0, 1, 4, 3, 2)
    ).astype(ml_dtypes.bfloat16)

    shared = {
        "w_enc": np.asarray(W_enc, dtype=np.float32).astype(ml_dtypes.bfloat16),
        "b_enc": np.asarray(b_enc, dtype=np.float32).reshape(A),
        "w_dec": np.asarray(W_dec, dtype=np.float32).astype(ml_dtypes.bfloat16),
        "b_dec": np.asarray(b_dec, dtype=np.float32).reshape(A),
        "w_att": np.asarray(W_att, dtype=np.float32).reshape(A),
    }
    in_maps = []
    for c in range(NCORES):
        # decT[p, hc*BLOC + pos] = dec[order[pos*NCORES+c], hc*P + p]
        bidx = [int(order[pos * NCORES + c]) for pos in range(BLOC)]
        decT = np.ascontiguousarray(
            decoder_hidden[bidx].reshape(BLOC, HID // P, P)
            .transpose(2, 1, 0).reshape(P, (HID // P) * BLOC)
        ).astype(ml_dtypes.bfloat16)
        in_maps.append({
            "encT": encT[c],
            "decT": decT,
            "masks": mask_t[c],
            **shared,
        })
    return in_maps, order, kps, kmins


def kernel(enc_output, decoder_hidden, masks, W_enc, b_enc, W_dec, b_dec,
           W_att, b_att, **kwargs):
    # b_att shifts every score equally -> cancels in softmax; output does not
    # depend on it, so it is not shipped to the device.
    in_maps, order, kps, kmins = host_prep(enc_output, decoder_hidden, masks,
                                           W_enc, b_enc, W_dec, b_dec, W_att,
                                           b_att)
    res = run_bass_kernel_spmd(_get_nc(kps, kmins), in_maps,
                               core_ids=list(range(NCORES)))
    stacked = np.concatenate([res.results[c]["out"] for c in range(NCORES)],
                             axis=0)  # [core*BLOC + pos]
    out = np.empty_like(stacked)
    for pos in range(BLOC):
        for core in range(NCORES):
            out[int(order[pos * NCORES + core])] = stacked[core * BLOC + pos]
    return out
